# revision 7
# baseline (speedup 1.0000x reference)
"""HGNNConv Trainium2 kernel, 8-core SPMD, two launches.

Math (linearity rearrangement — projection moved after aggregation):
  out = relu( S @ (X @ W + b) ),  S = Dv^-1/2 H De^-1 H^T Dv^-1/2
      = relu( (S @ X) @ W + (S @ 1) b^T )

Launch 1 (edge-sharded): Ye[e] = de_inv[e] * sum_{(v,e)} dv_isqrt[v] * X[v]
Launch 2 (vertex-sharded): A[v] = sum_{(v,e)} Ye[e];  out[v] = relu(dv_isqrt[v]*(A[v] @ W + s'[v] b^T))

Segment sums run on the tensor engine: gathered pair rows (dma_gather, bf16)
are reduced per 128-wide destination block via one-hot matmuls accumulated in
PSUM. One-hot built with iota + is_equal on DVE; stage-A per-pair dv_isqrt
weights applied to the one-hot by a per-partition-scaled ACT copy.
"""
import time
import numpy as np
import ml_dtypes
import concourse.bass as bass
import concourse.bacc as bacc
import concourse.mybir as mybir
from concourse.tile import TileContext
from concourse.masks import make_identity
from concourse import bass_utils

N, E, NNZ, C = 100000, 25000, 1600000, 256
NCORES = 8
P = 128

EPAD = 25600            # 200 edge blocks
NB_A = 25               # edge blocks per core
ESH = NB_A * P          # 3200 edges per core
NBANKS = 4
BANK = 25000            # X rows per bank (< int16 max)

NPADV = 100352          # 784 vertex blocks
NB_B = 98               # vertex blocks per core
VSH = NB_B * P          # 12544 vertices per core

f32 = mybir.dt.float32
bf16 = mybir.dt.bfloat16
i16 = mybir.dt.int16

_CACHE = {}


def _wrap16(idx_flat):
    """int16 gather indices: pos k -> [k%16, k//16], replicated to 128 partitions."""
    n = len(idx_flat)
    blk = np.zeros((16, n // 16), np.int16)
    blk[np.arange(n) % 16, np.arange(n) // 16] = idx_flat
    return np.tile(blk, (8, 1))


def _wrap128(a_flat, dtype=np.float32):
    n = len(a_flat)
    out = np.zeros((P, n // P), dtype)
    out[np.arange(n) % P, np.arange(n) // P] = a_flat
    return out


def _group_pairs(v_all, e_all, owner, j, key2, nkey2, dest_block, nblocks,
                 counts_max, src_idx, slot, wgt):
    """Build padded flat per-core arrays for one stage.

    counts_max[lb, k2] = padded chunk count (in pairs, multiple of 128).
    Returns (idx16, slotf, wgtf or None) flat arrays of total length sum(counts_max).
    """
    m = owner == j
    vb, eb = v_all[m], e_all[m]
    lb = dest_block[m]
    k2 = key2[m] if key2 is not None else np.zeros(lb.shape, np.int64)
    order = np.lexsort((k2, lb))
    vb, eb, lb, k2 = vb[order], eb[order], lb[order], k2[order]
    gid = lb * nkey2 + k2
    total = int(counts_max.sum())
    idx16 = np.zeros(total, np.int16)
    slotf = np.full(total, -1.0, np.float32)
    wgtf = np.zeros(total, np.float32) if wgt is not None else None
    # destination offset for each group
    offs = np.concatenate([[0], np.cumsum(counts_max.ravel())[:-1]])
    cnt = np.bincount(gid, minlength=nblocks * nkey2)
    # position of each pair within its group
    within = np.arange(len(gid)) - np.concatenate([[0], np.cumsum(cnt)[:-1]])[gid]
    dst = offs[gid] + within
    idx16[dst] = src_idx[m][order]
    slotf[dst] = slot[m][order]
    if wgtf is not None:
        wgtf[dst] = wgt[m][order]
    return idx16, slotf, wgtf


def _build_k1(CH_A):
    """Stage A kernel: gather X rows (bf16), weighted one-hot matmul -> Ye shard."""
    nc = bacc.Bacc("TRN2")
    total = int(CH_A.sum()) * P
    x = nc.dram_tensor("x", [N, C], bf16, kind="ExternalInput")
    idx = nc.dram_tensor("idx", [P, total // 16], i16, kind="ExternalInput")
    slot = nc.dram_tensor("slot", [P, total // P], f32, kind="ExternalInput")
    wgt = nc.dram_tensor("wgt", [P, total // P], f32, kind="ExternalInput")
    deinv = nc.dram_tensor("deinv", [P, NB_A], f32, kind="ExternalInput")
    ye = nc.dram_tensor("ye", [ESH, C], f32, kind="ExternalOutput")

    with TileContext(nc) as tc:
        with (
            tc.tile_pool(name="cpool", bufs=1) as cpool,
            tc.tile_pool(name="gpool", bufs=3) as gpool,
            tc.tile_pool(name="opool", bufs=3) as opool,
            tc.tile_pool(name="spool", bufs=3) as spool,
            tc.tile_pool(name="psum", bufs=4, space="PSUM") as psum_tp,
        ):
            iota_t = cpool.tile([P, P], f32)
            nc.gpsimd.iota(iota_t[:], pattern=[[1, P]], base=0,
                           channel_multiplier=0,
                           allow_small_or_imprecise_dtypes=True)
            idx_t = cpool.tile([P, total // 16], i16)
            nc.sync.dma_start(out=idx_t[:], in_=idx[:])
            slot_t = cpool.tile([P, total // P], f32)
            nc.sync.dma_start(out=slot_t[:], in_=slot[:])
            wgt_t = cpool.tile([P, total // P], f32)
            nc.sync.dma_start(out=wgt_t[:], in_=wgt[:])
            deinv_t = cpool.tile([P, NB_A], f32)
            nc.sync.dma_start(out=deinv_t[:], in_=deinv[:])

            gchunk = 0  # global chunk cursor
            for lb in range(NB_A):
                nch_blk = int(CH_A[lb].sum())
                acc = psum_tp.tile([P, C], f32, space="PSUM", tag="acc")
                ci = 0  # chunk index within block
                for bank in range(NBANKS):
                    nch = int(CH_A[lb, bank])
                    if nch == 0:
                        continue
                    for c0 in range(0, nch, 6):
                        cc = min(6, nch - c0)
                        gath = gpool.tile([P, cc, C], bf16, tag="gath")
                        nidx = cc * P
                        nc.gpsimd.dma_gather(
                            gath[:],
                            x[bank * BANK:(bank + 1) * BANK, :],
                            idx_t[:, (gchunk + c0) * 8:(gchunk + c0 + cc) * 8],
                            nidx, nidx, C,
                        )
                        for cL in range(cc):
                            c = c0 + cL
                            ohw = opool.tile([P, P], bf16, tag="ohw")
                            nc.vector.tensor_scalar(
                                out=ohw[:], in0=iota_t[:],
                                scalar1=slot_t[:, gchunk + c:gchunk + c + 1],
                                scalar2=wgt_t[:, gchunk + c:gchunk + c + 1],
                                op0=mybir.AluOpType.is_equal,
                                op1=mybir.AluOpType.mult,
                            )
                            nc.tensor.matmul(
                                out=acc[:], lhsT=ohw[:], rhs=gath[:, cL, :],
                                start=(ci == 0), stop=(ci == nch_blk - 1),
                            )
                            ci += 1
                    gchunk += nch
                out_t = spool.tile([P, C], f32, tag="out")
                nc.scalar.activation(
                    out=out_t[:], in_=acc[:],
                    func=mybir.ActivationFunctionType.Copy,
                    scale=deinv_t[:, lb:lb + 1],
                )
                nc.sync.dma_start(out=ye[lb * P:(lb + 1) * P, :], in_=out_t[:])
    nc.finalize()
    return nc


def _build_k2(CH_B):
    """Stage B kernel: gather Ye rows (bf16), one-hot matmul -> A block;
    then transpose, @W, + s' b^T, relu(dv_isqrt * .) -> Z shard."""
    nc = bacc.Bacc("TRN2")
    total = int(CH_B.sum()) * P
    yef = nc.dram_tensor("yef", [EPAD, C], bf16, kind="ExternalInput")
    idx = nc.dram_tensor("idx", [P, total // 16], i16, kind="ExternalInput")
    slot = nc.dram_tensor("slot", [P, total // P], f32, kind="ExternalInput")
    dvq = nc.dram_tensor("dvq", [P, NB_B], f32, kind="ExternalInput")
    sb = nc.dram_tensor("sb", [1, VSH], f32, kind="ExternalInput")
    w = nc.dram_tensor("w", [C, C], f32, kind="ExternalInput")
    bvec = nc.dram_tensor("bvec", [1, C], f32, kind="ExternalInput")
    z = nc.dram_tensor("z", [VSH, C], f32, kind="ExternalOutput")

    with TileContext(nc) as tc:
        with (
            tc.tile_pool(name="cpool", bufs=1) as cpool,
            tc.tile_pool(name="gpool", bufs=3) as gpool,
            tc.tile_pool(name="opool", bufs=3) as opool,
            tc.tile_pool(name="spool", bufs=3) as spool,
            tc.tile_pool(name="psum", bufs=2, space="PSUM") as psum_tp,
            tc.tile_pool(name="psumt", bufs=4, space="PSUM") as psumt_tp,
        ):
            iota_t = cpool.tile([P, P], f32)
            nc.gpsimd.iota(iota_t[:], pattern=[[1, P]], base=0,
                           channel_multiplier=0,
                           allow_small_or_imprecise_dtypes=True)
            ident = cpool.tile([P, P], f32)
            make_identity(nc, ident[:])
            idx_t = cpool.tile([P, total // 16], i16)
            nc.sync.dma_start(out=idx_t[:], in_=idx[:])
            slot_t = cpool.tile([P, total // P], f32)
            nc.sync.dma_start(out=slot_t[:], in_=slot[:])
            dvq_t = cpool.tile([P, NB_B], f32)
            nc.sync.dma_start(out=dvq_t[:], in_=dvq[:])
            sb_t = cpool.tile([1, VSH], f32)
            nc.sync.dma_start(out=sb_t[:], in_=sb[:])
            w_t = cpool.tile([P, 2, C], f32)
            nc.sync.dma_start(out=w_t[:, 0, :], in_=w[0:P, :])
            nc.sync.dma_start(out=w_t[:, 1, :], in_=w[P:C, :])
            b_t = cpool.tile([1, C], f32)
            nc.sync.dma_start(out=b_t[:], in_=bvec[:])

            gchunk = 0
            for lvb in range(NB_B):
                nch = int(CH_B[lvb])
                acc = psum_tp.tile([P, C], f32, space="PSUM", tag="acc")
                for c0 in range(0, nch, 6):
                    cc = min(6, nch - c0)
                    gath = gpool.tile([P, cc, C], bf16, tag="gath")
                    nidx = cc * P
                    nc.gpsimd.dma_gather(
                        gath[:], yef[:],
                        idx_t[:, (gchunk + c0) * 8:(gchunk + c0 + cc) * 8],
                        nidx, nidx, C,
                    )
                    for cL in range(cc):
                        c = c0 + cL
                        oh = opool.tile([P, P], bf16, tag="oh")
                        nc.vector.tensor_scalar(
                            out=oh[:], in0=iota_t[:],
                            scalar1=slot_t[:, gchunk + c:gchunk + c + 1],
                            scalar2=None,
                            op0=mybir.AluOpType.is_equal,
                        )
                        nc.tensor.matmul(
                            out=acc[:], lhsT=oh[:], rhs=gath[:, cL, :],
                            start=(c == 0), stop=(c == nch - 1),
                        )
                gchunk += nch
                # A block (f32) -> SBUF
                a_t = spool.tile([P, C], f32, tag="a")
                nc.scalar.activation(
                    out=a_t[:], in_=acc[:],
                    func=mybir.ActivationFunctionType.Copy,
                )
                # transpose both halves: [128, 128] each
                zacc = psum_tp.tile([P, C], f32, space="PSUM", tag="zacc")
                for h in range(2):
                    at_ps = psumt_tp.tile([P, P], f32, space="PSUM", tag="at")
                    nc.tensor.transpose(
                        out=at_ps[:], in_=a_t[:, h * P:(h + 1) * P], identity=ident[:],
                    )
                    at_sb = spool.tile([P, P], f32, tag="at_sb")
                    nc.scalar.activation(
                        out=at_sb[:], in_=at_ps[:],
                        func=mybir.ActivationFunctionType.Copy,
                    )
                    nc.tensor.matmul(
                        out=zacc[:], lhsT=at_sb[:], rhs=w_t[:, h, :],
                        start=(h == 0), stop=False,
                    )
                # + s'_block b^T  (rank-1, K=1)
                nc.tensor.matmul(
                    out=zacc[:], lhsT=sb_t[:, lvb * P:(lvb + 1) * P],
                    rhs=b_t[:], start=False, stop=True,
                )
                z_t = spool.tile([P, C], f32, tag="z")
                nc.scalar.activation(
                    out=z_t[:], in_=zacc[:],
                    func=mybir.ActivationFunctionType.Relu,
                    scale=dvq_t[:, lvb:lvb + 1],
                )
                nc.sync.dma_start(out=z[lvb * P:(lvb + 1) * P, :], in_=z_t[:])
    nc.finalize()
    return nc


def kernel(X, W, b, v_idx, e_idx):
    X = np.asarray(X, np.float32)
    W = np.asarray(W, np.float32)
    b = np.asarray(b, np.float32).reshape(-1)
    v = np.asarray(v_idx).astype(np.int64)
    e = np.asarray(e_idx).astype(np.int64)

    deg_v = np.bincount(v, minlength=N).astype(np.float64)
    deg_e = np.bincount(e, minlength=E).astype(np.float64)
    dv_isqrt = np.where(deg_v > 0, 1.0 / np.sqrt(np.maximum(deg_v, 1.0)), 0.0).astype(np.float32)
    de_inv = np.where(deg_e > 0, 1.0 / np.maximum(deg_e, 1.0), 0.0).astype(np.float32)

    # s' for the bias term: s'_v = sum_{e in v} de_inv[e] * t_e, t_e = sum dv_isqrt
    t_e = np.bincount(e, weights=dv_isqrt[v], minlength=E)
    s_p = np.bincount(v, weights=(de_inv * t_e)[e], minlength=N).astype(np.float32)

    X_bf = X.astype(ml_dtypes.bfloat16)

    # ---- stage A grouping (edge-sharded, 4 v-banks) ----
    eb = e // P
    ownerA = eb // NB_A
    lbA = eb - ownerA * NB_A
    bankA = v // BANK
    cntA = np.zeros((NCORES, NB_A, NBANKS), np.int64)
    np.add.at(cntA, (ownerA, lbA, bankA), 1)
    CH_A = (cntA.max(axis=0) + P - 1) // P          # [NB_A, NBANKS] chunks
    for lb in range(NB_A):
        if CH_A[lb].sum() == 0:
            CH_A[lb, 0] = 1
    cmaxA = (CH_A * P)

    # ---- stage B grouping (vertex-sharded) ----
    vb = v // P
    ownerB = vb // NB_B
    lvbB = vb - ownerB * NB_B
    cntB = np.zeros((NCORES, NB_B), np.int64)
    np.add.at(cntB, (ownerB, lvbB), 1)
    CH_B = np.maximum((cntB.max(axis=0) + P - 1) // P, 1)  # [NB_B]
    cmaxB = CH_B * P

    key = (CH_A.tobytes(), CH_B.tobytes())
    if key not in _CACHE:
        _CACHE[key] = (_build_k1(CH_A), _build_k2(CH_B))
    nc1, nc2 = _CACHE[key]

    # ---- per-core inputs, launch 1 ----
    in_maps1 = []
    for j in range(NCORES):
        idx16, slotf, wgtf = _group_pairs(
            v, e, ownerA, j, bankA, NBANKS, lbA, NB_A, cmaxA,
            src_idx=(v - bankA * BANK), slot=(e % P).astype(np.float32),
            wgt=dv_isqrt[v])
        deinv_cols = np.zeros((P, NB_A), np.float32)
        lo = j * ESH
        seg = de_inv[lo:min(lo + ESH, E)]
        segp = np.zeros(ESH, np.float32)
        segp[:len(seg)] = seg
        deinv_cols[:, :] = segp.reshape(NB_A, P).T
        in_maps1.append({
            "x": X_bf,
            "idx": _wrap16(idx16),
            "slot": _wrap128(slotf),
            "wgt": _wrap128(wgtf),
            "deinv": deinv_cols,
        })
    _t1 = time.time()
    res1 = bass_utils.run_bass_kernel_spmd(nc1, in_maps1, core_ids=list(range(NCORES)))
    _w1 = time.time() - _t1
    ye_full = np.concatenate([res1.results[j]["ye"] for j in range(NCORES)], axis=0)
    ye_bf = ye_full.astype(ml_dtypes.bfloat16)

    # ---- per-core inputs, launch 2 ----
    in_maps2 = []
    for j in range(NCORES):
        idx16, slotf, _ = _group_pairs(
            v, e, ownerB, j, None, 1, lvbB, NB_B, cmaxB.reshape(NB_B, 1),
            src_idx=e, slot=(v % P).astype(np.float32), wgt=None)
        dvq_cols = np.zeros((P, NB_B), np.float32)
        sb_row = np.zeros((1, VSH), np.float32)
        lo = j * VSH
        seg = dv_isqrt[lo:min(lo + VSH, N)]
        segp = np.zeros(VSH, np.float32)
        segp[:len(seg)] = seg
        dvq_cols[:, :] = segp.reshape(NB_B, P).T
        seg2 = s_p[lo:min(lo + VSH, N)]
        sb_row[0, :len(seg2)] = seg2
        in_maps2.append({
            "yef": ye_bf,
            "idx": _wrap16(idx16),
            "slot": _wrap128(slotf),
            "dvq": dvq_cols,
            "sb": sb_row,
            "w": W,
            "bvec": b.reshape(1, C),
        })
    _t2 = time.time()
    res2 = bass_utils.run_bass_kernel_spmd(nc2, in_maps2, core_ids=list(range(NCORES)))
    _w2 = time.time() - _t2
    kernel._last_wall = (_w1, _w2)
    z = np.concatenate([res2.results[j]["z"] for j in range(NCORES)], axis=0)
    kernel._last_exec_ns = (res1.exec_time_ns, res2.exec_time_ns)
    return z[:N]
